# revision 35
# baseline (speedup 1.0000x reference)
"""CrossModalAttention kernel for 8x TRN2 NeuronCores (batch data-parallel).

Reference computation (per batch element b, context input is unused):
    qkv = x @ qkv_w + qkv_b            # [N, 3C]
    q, k, v = split(qkv)               # heads H=12, d=64
    attn = softmax(q*scale @ k^T)      # per head, N=1024
    out = (attn @ v) @ proj_w + proj_b # [N, C]

Strategy per core (one batch element each). v3: single flat scope so the
Tile list-scheduler can interleave the qkv/v/proj GEMMs as PE "filler"
between attention iterations — the PE<->ACT exp ping-pong otherwise
leaves the PE idle in ~1-5us gaps, which trips the HAM activity
throttle to half clock. Keeping the PE saturated holds 2.4GHz.

  - All matmul operands fp16 (PSUM accumulates fp32; TF32-equivalent
    mantissa, measured rel err ~5e-4 vs 2e-2 budget). Halves input DMA
    and SBUF, and enables fast weight load on the PE.
  - Host pre-transposes x -> xT [C, N], splits qkv_w into wqk (q part
    pre-scaled by d^-0.5) and wv.
  - qkT [2*C, N] fp16 (transposed: per-head qT/kT slices directly feed
    the scores matmuls). v natural [N, C] fp16 augmented with a ones
    column per head (v_aug [N, H*65]) so attn@v also produces softmax
    denominators.
  - Per head pair (2 heads in the 128 partitions): scoresT[k, q] =
    kT-as-lhsT @ qT (K=64 halves), exp on ACT (scores O(1), no max
    subtraction), attn@v accumulated over k chunks into psum [65, N]:
    rows 0-63 = out^T, row 64 = sums. 1/sums via DVE fast reciprocal
    (from an SBUF copy: reading PSUM directly raced the matmul drain on
    HW and produced NaNs), partition-broadcast via DRAM round trip,
    normalize with DVE into fp16 outT.
  - proj: final[t, :] = outT-as-lhsT @ wproj, natural layout, DMA out.
  PSUM budget (8 banks): sc ring 2x[128,1024] = 4 (also carries the
  qkv/v/proj filler groups), av A+B 2x[65,1024] = 4.
"""
import numpy as np

import concourse.bass as bass
import concourse.tile as tile
from concourse import bacc, mybir
from concourse.bass_utils import run_bass_kernel_spmd

DIM = 768
NUM_HEADS = 12
HEAD_DIM = 64
B, N = 8, 1024
P = 128
KC = DIM // P          # 6 contraction chunks of 128 over channels
TC = N // P            # 8 token chunks of 128
QC = N // 512          # 2 free-dim chunks of 512 over tokens
HP = NUM_HEADS // 2    # 6 head pairs
VAUG = 65              # v columns per head: 64 v dims + 1 ones column

F32 = mybir.dt.float32
F32R = mybir.dt.float32r
FP16 = mybir.dt.float16


def build_nc(with_qkv_bias: bool, with_proj_bias: bool):
    nc = bacc.Bacc("TRN2", target_bir_lowering=False, debug=False)

    xT_d = nc.dram_tensor("xT", [DIM, N], FP16, kind="ExternalInput")
    wqk_d = nc.dram_tensor("wqk", [DIM, 2 * DIM], FP16, kind="ExternalInput")
    wv_d = nc.dram_tensor("wv", [DIM, DIM], FP16, kind="ExternalInput")
    wproj_d = nc.dram_tensor("wproj", [DIM, DIM], FP16, kind="ExternalInput")
    bqk_d = nc.dram_tensor("bqk", [1, 2 * DIM], F32, kind="ExternalInput")
    bv_d = nc.dram_tensor("bv", [1, DIM], F32, kind="ExternalInput")
    bproj_d = nc.dram_tensor("bproj", [1, DIM], F32, kind="ExternalInput")
    out_d = nc.dram_tensor("out", [N, DIM], FP16, kind="ExternalOutput")

    with tile.TileContext(nc) as tc:
        with (
            tc.tile_pool(name="consts", bufs=1) as consts,
            tc.tile_pool(name="inputs", bufs=1) as in_pool,
            tc.tile_pool(name="qk_sb", bufs=1) as qk_pool,
            tc.tile_pool(name="vaug_sb", bufs=1) as vaug_pool,
            tc.tile_pool(name="outT_sb", bufs=1) as outT_pool,
            tc.tile_pool(name="expT", bufs=6) as exp_pool,
            tc.tile_pool(name="norm", bufs=4) as norm_pool,
            tc.tile_pool(name="rep", bufs=3) as rep_pool,
            tc.tile_pool(name="fin", bufs=3) as fin_pool,
            tc.tile_pool(name="partial", bufs=1) as partial_pool,
            tc.tile_pool(name="dramp", bufs=1, space="DRAM") as dram_pool,
            tc.tile_pool(name="ps_sc", bufs=2, space="PSUM") as ps_sc,
            tc.tile_pool(name="ps_av", bufs=2, space="PSUM") as ps_av,
        ):
            # ---- ACT exp table preload: a dummy exp so the ~2.7us
            # ACT_TABLE_LOAD happens during the input DMA wait ----
            warm_in = consts.tile([1, 16], F32)
            nc.vector.memset(warm_in[:], 0.0)
            warm_out = consts.tile([1, 16], FP16)
            nc.scalar.activation(
                warm_out[:], warm_in[:], mybir.ActivationFunctionType.Exp
            )

            # ---- constants ----
            if with_qkv_bias or with_proj_bias:
                ones_f32 = consts.tile([1, N], F32)
                nc.vector.memset(ones_f32[:], 1.0)
                ones_row = consts.tile([1, N], FP16)
                nc.vector.tensor_copy(ones_row[:], ones_f32[:])
            if with_qkv_bias:
                bqk_f32 = consts.tile([1, 2 * DIM], F32)
                nc.sync.dma_start(out=bqk_f32[:], in_=bqk_d[:])
                bqk_sb = consts.tile([1, 2 * DIM], FP16)
                nc.vector.tensor_copy(bqk_sb[:], bqk_f32[:])
                bv_f32 = consts.tile([1, DIM], F32)
                nc.sync.dma_start(out=bv_f32[:], in_=bv_d[:])
                bv_sb = consts.tile([1, DIM], FP16)
                nc.vector.tensor_copy(bv_sb[:], bv_f32[:])
            if with_proj_bias:
                bproj_f32 = consts.tile([1, DIM], F32)
                nc.sync.dma_start(out=bproj_f32[:], in_=bproj_d[:])
                bproj_sb = consts.tile([1, DIM], FP16)
                nc.vector.tensor_copy(bproj_sb[:], bproj_f32[:])

            # ---- input DMAs (wqk/xT interleaved so qkT m=0 accumulates
            # as chunks land; wv next, wproj last) ----
            xT = [in_pool.tile([P, N], FP16, name=f"xT{c}") for c in range(KC)]
            wqk = [
                in_pool.tile([P, 2 * DIM], FP16, name=f"wqk{c}") for c in range(KC)
            ]
            wv = [in_pool.tile([P, DIM], FP16, name=f"wv{c}") for c in range(KC)]
            wproj = [
                in_pool.tile([P, DIM], FP16, name=f"wproj{c}") for c in range(KC)
            ]
            for c in range(KC):
                sl = slice(c * P, (c + 1) * P)
                nc.sync.dma_start(out=xT[c][:], in_=xT_d[sl, :])
                nc.sync.dma_start(out=wqk[c][:], in_=wqk_d[sl, :])
            for c in range(KC):
                sl = slice(c * P, (c + 1) * P)
                nc.sync.dma_start(out=wv[c][:], in_=wv_d[sl, :])

            # ---- persistent tiles ----
            qkT = [
                qk_pool.tile([P, N], FP16, name=f"qkT{m}") for m in range(2 * KC)
            ]  # m-chunk m covers qkv channels m*128..m*128+127 (q then k)
            v_aug = [
                vaug_pool.tile([P, NUM_HEADS * VAUG], FP16, name=f"vaug{t}")
                for t in range(TC)
            ]
            outT = [
                outT_pool.tile([P, N], FP16, name=f"outT{p}") for p in range(HP)
            ]  # pair p: head 2p in parts 0-63, head 2p+1 in parts 64-127
            recip_d = dram_pool.tile([NUM_HEADS, N], F32)

            # ---- filler emitters: qkv/v/proj matmul groups the scheduler
            # slots into PE idle gaps during attention. qkT fillers come in
            # half-token-range units to limit sc-ring slot hold time. ----
            def emit_qkT(m, q=None):
                qs = range(QC) if q is None else [q]
                width = N if q is None else 512
                ps = ps_sc.tile([P, width], F32, name=f"ps_qk{m}_{qs[0]}", tag="sc")
                msl = slice(m * P, (m + 1) * P)
                for c in range(KC):
                    for qi, qq in enumerate(qs):
                        qsl = slice(qq * 512, (qq + 1) * 512)
                        osl = slice(qi * 512, (qi + 1) * 512)
                        nc.tensor.matmul(
                            ps[:, osl],
                            wqk[c][:, msl],
                            xT[c][:, qsl],
                            start=(c == 0),
                            stop=(c == KC - 1) and not with_qkv_bias,
                        )
                if with_qkv_bias:
                    for qi, qq in enumerate(qs):
                        qsl = slice(qq * 512, (qq + 1) * 512)
                        osl = slice(qi * 512, (qi + 1) * 512)
                        nc.tensor.matmul(
                            ps[:, osl],
                            bqk_sb[:, msl],
                            ones_row[:, qsl],
                            start=False,
                            stop=True,
                        )
                for qi, qq in enumerate(qs):
                    qsl = slice(qq * 512, (qq + 1) * 512)
                    osl = slice(qi * 512, (qi + 1) * 512)
                    nc.vector.tensor_copy(qkT[m][:, qsl], ps[:, osl])

            def emit_v(t):
                ps = ps_sc.tile([P, DIM], F32, name=f"ps_v{t}", tag="sc")
                tsl = slice(t * P, (t + 1) * P)
                for c in range(KC):
                    for nsl in (slice(0, 512), slice(512, DIM)):
                        nc.tensor.matmul(
                            ps[:, nsl],
                            xT[c][:, tsl],
                            wv[c][:, nsl],
                            start=(c == 0),
                            stop=(c == KC - 1) and not with_qkv_bias,
                        )
                if with_qkv_bias:
                    for nsl in (slice(0, 512), slice(512, DIM)):
                        nc.tensor.matmul(
                            ps[:, nsl],
                            ones_row[:, t * P : t * P + P],
                            bv_sb[:, nsl],
                            start=False,
                            stop=True,
                        )
                va3 = v_aug[t][:].rearrange("p (h e) -> p h e", e=VAUG)
                nc.vector.memset(va3[:, :, 64:65], 1.0)
                nc.vector.tensor_copy(
                    va3[:, :, 0:64],
                    ps[:].rearrange("p (h d) -> p h d", d=HEAD_DIM),
                )

            # proj is split: c=0..4 accumulate into SBUF partials (usable
            # as PE filler during pair 5, before outT[5] exists); the c=5
            # matmul + DVE add finish after pair 5's normalization.
            partials = [
                partial_pool.tile([P, DIM], F32, name=f"pjpart{t}")
                for t in range(TC)
            ]

            def emit_proj_partial(t):
                ps = ps_sc.tile([P, DIM], F32, name=f"pjp{t}", tag="sc")
                tsl = slice(t * P, (t + 1) * P)
                for c in range(KC - 1):
                    for nsl in (slice(0, 512), slice(512, DIM)):
                        nc.tensor.matmul(
                            ps[:, nsl],
                            outT[c][:, tsl],
                            wproj[c][:, nsl],
                            start=(c == 0),
                            stop=(c == KC - 2),
                        )
                nc.vector.tensor_copy(partials[t][:], ps[:])

            def emit_proj_finish(t):
                ps = ps_sc.tile([P, DIM], F32, name=f"pjf{t}", tag="sc")
                tsl = slice(t * P, (t + 1) * P)
                for nsl in (slice(0, 512), slice(512, DIM)):
                    nc.tensor.matmul(
                        ps[:, nsl],
                        outT[KC - 1][:, tsl],
                        wproj[KC - 1][:, nsl],
                        start=True,
                        stop=not with_proj_bias,
                    )
                if with_proj_bias:
                    for nsl in (slice(0, 512), slice(512, DIM)):
                        nc.tensor.matmul(
                            ps[:, nsl],
                            ones_row[:, t * P : t * P + P],
                            bproj_sb[:, nsl],
                            start=False,
                            stop=True,
                        )
                fin = fin_pool.tile([P, DIM], FP16, name=f"fin{t}", tag="fin")
                nc.vector.tensor_tensor(
                    out=fin[:], in0=ps[:], in1=partials[t][:],
                    op=mybir.AluOpType.add,
                )
                nc.sync.dma_start(out=out_d[tsl, :], in_=fin[:])

            # Filler schedule, per head now. Legality: head 2j needs q
            # chunk m=j complete and kT chunk m=6+j half0 by kc0, half1 by
            # kc4 (kT columns kc*128 are only read at AV step kc).
            # v_aug[kc] is consumed at every head's AV step kc, so v3..v7
            # must materialize during head 0. Heads 10/11 run proj c=0..4
            # partials (outT[0..4] exist by then).
            fillers = [[] for _ in range(NUM_HEADS)]
            fillers[0] = [
                (0, emit_v, (3,)), (1, emit_v, (4,)), (2, emit_v, (5,)),
                (3, emit_v, (6,)), (4, emit_v, (7,)),
                (5, emit_qkT, (1, 0)), (6, emit_qkT, (1, 1)),
                (7, emit_qkT, (KC + 1, 0)),
            ]
            fillers[1] = [(0, emit_qkT, (KC + 1, 1))]
            for j in range(2, KC):
                fillers[2 * (j - 1)] += [
                    (2, emit_qkT, (j, 0)), (5, emit_qkT, (j, 1)),
                ]
                fillers[2 * (j - 1) + 1] += [
                    (2, emit_qkT, (KC + j, 0)), (5, emit_qkT, (KC + j, 1)),
                ]
            fillers[10] += [(5, emit_proj_partial, (0,)), (7, emit_proj_partial, (1,))]
            fillers[11] += [
                (1, emit_proj_partial, (2,)), (3, emit_proj_partial, (3,)),
                (5, emit_proj_partial, (4,)),
            ]

            # ---- prologue: head 0/1 operands (dense PE work during the
            # input DMA stream keeps the clock ramping) ----
            emit_qkT(0)
            emit_qkT(KC)
            emit_v(0)
            emit_v(1)
            emit_v(2)

            # ---- attention: 12 heads with inline filler. av bufs=2 means
            # head h+1 accumulates while head h's normalization drains —
            # no PSUM-slot stall at head boundaries. ----
            for h in range(NUM_HEADS):
                hrow = slice((h % 2) * 64, (h % 2) * 64 + 64)
                qT = qkT[h // 2]
                kT = qkT[KC + h // 2]
                av = ps_av.tile([VAUG, N], F32, name=f"av{h}", tag="av")
                for kc in range(TC):
                    ksl = slice(kc * P, (kc + 1) * P)
                    sc = ps_sc.tile([P, N], F32, name=f"sc{h}_{kc}", tag="sc")
                    for q in range(QC):
                        qsl = slice(q * 512, (q + 1) * 512)
                        nc.tensor.matmul(
                            sc[:, qsl], kT[hrow, ksl], qT[hrow, qsl],
                            start=True, stop=True,
                        )
                    eT = exp_pool.tile([P, N], FP16, name=f"e{h}_{kc}", tag="e")
                    nc.scalar.activation(
                        eT[:], sc[:], mybir.ActivationFunctionType.Exp
                    )
                    for q in range(QC):
                        qsl = slice(q * 512, (q + 1) * 512)
                        nc.tensor.matmul(
                            av[:, qsl],
                            v_aug[kc][:, h * VAUG : (h + 1) * VAUG],
                            eT[:, qsl],
                            start=(kc == 0), stop=(kc == TC - 1),
                        )
                    for fkc, fn, args in fillers[h]:
                        if fkc == kc:
                            fn(*args)
                if h == 8:
                    # wproj can land any time before the proj partials
                    for c in range(KC):
                        nc.sync.dma_start(
                            out=wproj[c][:],
                            in_=wproj_d[c * P : (c + 1) * P, :],
                        )
                # ---- normalization: sums -> 1/sums -> broadcast across
                # partitions via DRAM round trip -> DVE multiply ----
                sums_t = norm_pool.tile([1, N], F32, name=f"sums{h}", tag="sums")
                recip_t = norm_pool.tile([1, N], F32, name=f"recip{h}", tag="recip")
                nc.vector.tensor_copy(sums_t[:], av[64:65, :])
                nc.vector.reciprocal_approx_fast(out=recip_t[:], in_=sums_t[:])
                nc.sync.dma_start(out=recip_d[h : h + 1, :], in_=recip_t[:])
                rep = rep_pool.tile([64, N], F32, name=f"rep{h}", tag="rep")
                nc.sync.dma_start(
                    out=rep[:],
                    in_=recip_d[h : h + 1, :].to_broadcast([64, N]),
                )
                nc.vector.tensor_tensor(
                    out=outT[h // 2][hrow, :],
                    in0=av[0:64, :],
                    in1=rep[:],
                    op=mybir.AluOpType.mult,
                )

            # ---- epilogue: remaining proj partials bridge the last
            # head's normalization latency, then the c=5 finishes ----
            for t in range(TC - 3, TC):
                emit_proj_partial(t)
            for t in range(TC):
                emit_proj_finish(t)

    nc.compile()
    return nc


_NC_CACHE = {}


def kernel(**inputs) -> np.ndarray:
    x = np.asarray(inputs["x"], dtype=np.float32)
    qkv_w = np.asarray(inputs["qkv_w"], dtype=np.float32)
    qkv_b = np.asarray(inputs["qkv_b"], dtype=np.float32)
    proj_w = np.asarray(inputs["proj_w"], dtype=np.float32)
    proj_b = np.asarray(inputs["proj_b"], dtype=np.float32)
    # context is unused by the reference layer.

    scale = HEAD_DIM ** -0.5
    wqk = qkv_w[:, : 2 * DIM].copy()
    wqk[:, :DIM] *= scale
    wv = np.ascontiguousarray(qkv_w[:, 2 * DIM :])
    bqk = qkv_b[: 2 * DIM].copy()
    bqk[:DIM] *= scale
    bv = qkv_b[2 * DIM :].copy()

    with_qkv_bias = bool(np.any(qkv_b))
    with_proj_bias = bool(np.any(proj_b))

    key = (with_qkv_bias, with_proj_bias)
    if key not in _NC_CACHE:
        _NC_CACHE[key] = build_nc(*key)
    nc = _NC_CACHE[key]

    base = {
        "wqk": wqk.astype(np.float16),
        "wv": wv.astype(np.float16),
        "wproj": proj_w.astype(np.float16),
        "bqk": bqk.reshape(1, -1),
        "bv": bv.reshape(1, -1),
        "bproj": proj_b.reshape(1, -1),
    }
    in_maps = [
        {**base, "xT": np.ascontiguousarray(x[b].T).astype(np.float16)}
        for b in range(B)
    ]
    res = run_bass_kernel_spmd(nc, in_maps, list(range(B)))
    out = np.stack([res.results[b]["out"] for b in range(B)], axis=0)
    return out.astype(np.float32)


# revision 36
# speedup vs baseline: 1.0367x; 1.0367x over previous
"""CrossModalAttention kernel for 8x TRN2 NeuronCores (batch data-parallel).

Reference computation (per batch element b, context input is unused):
    qkv = x @ qkv_w + qkv_b            # [N, 3C]
    q, k, v = split(qkv)               # heads H=12, d=64
    attn = softmax(q*scale @ k^T)      # per head, N=1024
    out = (attn @ v) @ proj_w + proj_b # [N, C]

Strategy per core (one batch element each). v3: single flat scope so the
Tile list-scheduler can interleave the qkv/v/proj GEMMs as PE "filler"
between attention iterations — the PE<->ACT exp ping-pong otherwise
leaves the PE idle in ~1-5us gaps, which trips the HAM activity
throttle to half clock. Keeping the PE saturated holds 2.4GHz.

  - All matmul operands fp16 (PSUM accumulates fp32; TF32-equivalent
    mantissa, measured rel err ~5e-4 vs 2e-2 budget). Halves input DMA
    and SBUF, and enables fast weight load on the PE.
  - Host pre-transposes x -> xT [C, N], splits qkv_w into wqk (q part
    pre-scaled by d^-0.5) and wv.
  - qkT [2*C, N] fp16 (transposed: per-head qT/kT slices directly feed
    the scores matmuls). v natural [N, C] fp16 augmented with a ones
    column per head (v_aug [N, H*65]) so attn@v also produces softmax
    denominators.
  - Per head pair (2 heads in the 128 partitions): scoresT[k, q] =
    kT-as-lhsT @ qT (K=64 halves), exp on ACT (scores O(1), no max
    subtraction), attn@v accumulated over k chunks into psum [65, N]:
    rows 0-63 = out^T, row 64 = sums. 1/sums via DVE fast reciprocal
    (from an SBUF copy: reading PSUM directly raced the matmul drain on
    HW and produced NaNs), partition-broadcast via DRAM round trip,
    normalize with DVE into fp16 outT.
  - proj: final[t, :] = outT-as-lhsT @ wproj, natural layout, DMA out.
  PSUM budget (8 banks): sc ring 2x[128,1024] = 4 (also carries the
  qkv/v/proj filler groups), av A+B 2x[65,1024] = 4.
"""
import numpy as np

import concourse.bass as bass
import concourse.tile as tile
from concourse import bacc, mybir
from concourse.bass_utils import run_bass_kernel_spmd

DIM = 768
NUM_HEADS = 12
HEAD_DIM = 64
B, N = 8, 1024
P = 128
KC = DIM // P          # 6 contraction chunks of 128 over channels
TC = N // P            # 8 token chunks of 128
QC = N // 512          # 2 free-dim chunks of 512 over tokens
HP = NUM_HEADS // 2    # 6 head pairs
VAUG = 65              # v columns per head: 64 v dims + 1 ones column

F32 = mybir.dt.float32
F32R = mybir.dt.float32r
FP16 = mybir.dt.float16


def build_nc(with_qkv_bias: bool, with_proj_bias: bool):
    nc = bacc.Bacc("TRN2", target_bir_lowering=False, debug=False)

    xT_d = nc.dram_tensor("xT", [DIM, N], FP16, kind="ExternalInput")
    wqk_d = nc.dram_tensor("wqk", [DIM, 2 * DIM], FP16, kind="ExternalInput")
    wv_d = nc.dram_tensor("wv", [DIM, DIM], FP16, kind="ExternalInput")
    wproj_d = nc.dram_tensor("wproj", [DIM, DIM], FP16, kind="ExternalInput")
    bqk_d = nc.dram_tensor("bqk", [1, 2 * DIM], F32, kind="ExternalInput")
    bv_d = nc.dram_tensor("bv", [1, DIM], F32, kind="ExternalInput")
    bproj_d = nc.dram_tensor("bproj", [1, DIM], F32, kind="ExternalInput")
    out_d = nc.dram_tensor("out", [N, DIM], FP16, kind="ExternalOutput")

    with tile.TileContext(nc) as tc:
        with (
            tc.tile_pool(name="consts", bufs=1) as consts,
            tc.tile_pool(name="inputs", bufs=1) as in_pool,
            tc.tile_pool(name="qk_sb", bufs=1) as qk_pool,
            tc.tile_pool(name="vaug_sb", bufs=1) as vaug_pool,
            tc.tile_pool(name="outT_sb", bufs=1) as outT_pool,
            tc.tile_pool(name="expT", bufs=8) as exp_pool,
            tc.tile_pool(name="norm", bufs=4) as norm_pool,
            tc.tile_pool(name="rep", bufs=4) as rep_pool,
            tc.tile_pool(name="fin", bufs=4) as fin_pool,
            tc.tile_pool(name="partial", bufs=1) as partial_pool,
            tc.tile_pool(name="dramp", bufs=1, space="DRAM") as dram_pool,
            tc.tile_pool(name="ps_sc", bufs=2, space="PSUM") as ps_sc,
            tc.tile_pool(name="ps_av", bufs=2, space="PSUM") as ps_av,
        ):
            # ---- ACT exp table preload: a dummy exp so the ~2.7us
            # ACT_TABLE_LOAD happens during the input DMA wait ----
            warm_in = consts.tile([1, 16], F32)
            nc.vector.memset(warm_in[:], 0.0)
            warm_out = consts.tile([1, 16], FP16)
            nc.scalar.activation(
                warm_out[:], warm_in[:], mybir.ActivationFunctionType.Exp
            )

            # ---- constants ----
            if with_qkv_bias or with_proj_bias:
                ones_f32 = consts.tile([1, N], F32)
                nc.vector.memset(ones_f32[:], 1.0)
                ones_row = consts.tile([1, N], FP16)
                nc.vector.tensor_copy(ones_row[:], ones_f32[:])
            if with_qkv_bias:
                bqk_f32 = consts.tile([1, 2 * DIM], F32)
                nc.sync.dma_start(out=bqk_f32[:], in_=bqk_d[:])
                bqk_sb = consts.tile([1, 2 * DIM], FP16)
                nc.vector.tensor_copy(bqk_sb[:], bqk_f32[:])
                bv_f32 = consts.tile([1, DIM], F32)
                nc.sync.dma_start(out=bv_f32[:], in_=bv_d[:])
                bv_sb = consts.tile([1, DIM], FP16)
                nc.vector.tensor_copy(bv_sb[:], bv_f32[:])
            if with_proj_bias:
                bproj_f32 = consts.tile([1, DIM], F32)
                nc.sync.dma_start(out=bproj_f32[:], in_=bproj_d[:])
                bproj_sb = consts.tile([1, DIM], FP16)
                nc.vector.tensor_copy(bproj_sb[:], bproj_f32[:])

            # ---- input DMAs (wqk/xT interleaved so qkT m=0 accumulates
            # as chunks land; wv next, wproj last) ----
            xT = [in_pool.tile([P, N], FP16, name=f"xT{c}") for c in range(KC)]
            wqk = [
                in_pool.tile([P, 2 * DIM], FP16, name=f"wqk{c}") for c in range(KC)
            ]
            wv = [in_pool.tile([P, DIM], FP16, name=f"wv{c}") for c in range(KC)]
            wproj = [
                in_pool.tile([P, DIM], FP16, name=f"wproj{c}") for c in range(KC)
            ]
            for c in range(KC):
                sl = slice(c * P, (c + 1) * P)
                nc.sync.dma_start(out=xT[c][:], in_=xT_d[sl, :])
                nc.sync.dma_start(out=wqk[c][:], in_=wqk_d[sl, :])
            for c in range(KC):
                sl = slice(c * P, (c + 1) * P)
                nc.sync.dma_start(out=wv[c][:], in_=wv_d[sl, :])

            # ---- persistent tiles ----
            qkT = [
                qk_pool.tile([P, N], FP16, name=f"qkT{m}") for m in range(2 * KC)
            ]  # m-chunk m covers qkv channels m*128..m*128+127 (q then k)
            v_aug = [
                vaug_pool.tile([P, NUM_HEADS * VAUG], FP16, name=f"vaug{t}")
                for t in range(TC)
            ]
            outT = [
                outT_pool.tile([P, N], FP16, name=f"outT{p}") for p in range(HP)
            ]  # pair p: head 2p in parts 0-63, head 2p+1 in parts 64-127
            recip_d = dram_pool.tile([NUM_HEADS, N], F32)

            # ---- filler emitters: qkv/v/proj matmul groups the scheduler
            # slots into PE idle gaps during attention. qkT fillers come in
            # half-token-range units to limit sc-ring slot hold time. ----
            def emit_qkT(m, q=None):
                qs = range(QC) if q is None else [q]
                width = N if q is None else 512
                ps = ps_sc.tile([P, width], F32, name=f"ps_qk{m}_{qs[0]}", tag="sc")
                msl = slice(m * P, (m + 1) * P)
                for c in range(KC):
                    for qi, qq in enumerate(qs):
                        qsl = slice(qq * 512, (qq + 1) * 512)
                        osl = slice(qi * 512, (qi + 1) * 512)
                        nc.tensor.matmul(
                            ps[:, osl],
                            wqk[c][:, msl],
                            xT[c][:, qsl],
                            start=(c == 0),
                            stop=(c == KC - 1) and not with_qkv_bias,
                        )
                if with_qkv_bias:
                    for qi, qq in enumerate(qs):
                        qsl = slice(qq * 512, (qq + 1) * 512)
                        osl = slice(qi * 512, (qi + 1) * 512)
                        nc.tensor.matmul(
                            ps[:, osl],
                            bqk_sb[:, msl],
                            ones_row[:, qsl],
                            start=False,
                            stop=True,
                        )
                for qi, qq in enumerate(qs):
                    qsl = slice(qq * 512, (qq + 1) * 512)
                    osl = slice(qi * 512, (qi + 1) * 512)
                    nc.vector.tensor_copy(qkT[m][:, qsl], ps[:, osl])

            def emit_v(t):
                ps = ps_sc.tile([P, DIM], F32, name=f"ps_v{t}", tag="sc")
                tsl = slice(t * P, (t + 1) * P)
                for c in range(KC):
                    for nsl in (slice(0, 512), slice(512, DIM)):
                        nc.tensor.matmul(
                            ps[:, nsl],
                            xT[c][:, tsl],
                            wv[c][:, nsl],
                            start=(c == 0),
                            stop=(c == KC - 1) and not with_qkv_bias,
                        )
                if with_qkv_bias:
                    for nsl in (slice(0, 512), slice(512, DIM)):
                        nc.tensor.matmul(
                            ps[:, nsl],
                            ones_row[:, t * P : t * P + P],
                            bv_sb[:, nsl],
                            start=False,
                            stop=True,
                        )
                va3 = v_aug[t][:].rearrange("p (h e) -> p h e", e=VAUG)
                nc.vector.memset(va3[:, :, 64:65], 1.0)
                nc.vector.tensor_copy(
                    va3[:, :, 0:64],
                    ps[:].rearrange("p (h d) -> p h d", d=HEAD_DIM),
                )

            # proj is split: c=0..4 accumulate into SBUF partials (usable
            # as PE filler during pair 5, before outT[5] exists); the c=5
            # matmul + DVE add finish after pair 5's normalization.
            partials = [
                partial_pool.tile([P, DIM], F32, name=f"pjpart{t}")
                for t in range(TC)
            ]

            def emit_proj_partial(t):
                ps = ps_sc.tile([P, DIM], F32, name=f"pjp{t}", tag="sc")
                tsl = slice(t * P, (t + 1) * P)
                for c in range(KC - 1):
                    for nsl in (slice(0, 512), slice(512, DIM)):
                        nc.tensor.matmul(
                            ps[:, nsl],
                            outT[c][:, tsl],
                            wproj[c][:, nsl],
                            start=(c == 0),
                            stop=(c == KC - 2),
                        )
                nc.vector.tensor_copy(partials[t][:], ps[:])

            def emit_proj_finish(t):
                ps = ps_sc.tile([P, DIM], F32, name=f"pjf{t}", tag="sc")
                tsl = slice(t * P, (t + 1) * P)
                for nsl in (slice(0, 512), slice(512, DIM)):
                    nc.tensor.matmul(
                        ps[:, nsl],
                        outT[KC - 1][:, tsl],
                        wproj[KC - 1][:, nsl],
                        start=True,
                        stop=not with_proj_bias,
                    )
                if with_proj_bias:
                    for nsl in (slice(0, 512), slice(512, DIM)):
                        nc.tensor.matmul(
                            ps[:, nsl],
                            ones_row[:, t * P : t * P + P],
                            bproj_sb[:, nsl],
                            start=False,
                            stop=True,
                        )
                fin = fin_pool.tile([P, DIM], FP16, name=f"fin{t}", tag="fin")
                nc.vector.tensor_tensor(
                    out=fin[:], in0=ps[:], in1=partials[t][:],
                    op=mybir.AluOpType.add,
                )
                nc.sync.dma_start(out=out_d[tsl, :], in_=fin[:])

            # Filler schedule, per head now. Legality: head 2j needs q
            # chunk m=j complete and kT chunk m=6+j half0 by kc0, half1 by
            # kc4 (kT columns kc*128 are only read at AV step kc).
            # v_aug[kc] is consumed at every head's AV step kc, so v3..v7
            # must materialize during head 0. Heads 10/11 run proj c=0..4
            # partials (outT[0..4] exist by then).
            fillers = [[] for _ in range(NUM_HEADS)]
            fillers[0] = [
                (0, emit_v, (3,)), (1, emit_v, (4,)), (2, emit_v, (5,)),
                (3, emit_v, (6,)), (4, emit_v, (7,)),
                (5, emit_qkT, (1, 0)), (6, emit_qkT, (1, 1)),
                (7, emit_qkT, (KC + 1, 0)),
            ]
            fillers[1] = [(0, emit_qkT, (KC + 1, 1))]
            for j in range(2, KC):
                fillers[2 * (j - 1)] += [
                    (2, emit_qkT, (j, 0)), (5, emit_qkT, (j, 1)),
                ]
                fillers[2 * (j - 1) + 1] += [
                    (2, emit_qkT, (KC + j, 0)), (5, emit_qkT, (KC + j, 1)),
                ]
            fillers[10] += [(5, emit_proj_partial, (0,)), (7, emit_proj_partial, (1,))]
            fillers[11] += [
                (1, emit_proj_partial, (2,)), (3, emit_proj_partial, (3,)),
                (5, emit_proj_partial, (4,)),
            ]

            # ---- prologue: head 0/1 operands (dense PE work during the
            # input DMA stream keeps the clock ramping) ----
            emit_qkT(0)
            emit_qkT(KC)
            emit_v(0)
            emit_v(1)
            emit_v(2)

            # ---- attention: 12 heads with inline filler. av bufs=2 means
            # head h+1 accumulates while head h's normalization drains —
            # no PSUM-slot stall at head boundaries. ----
            for h in range(NUM_HEADS):
                hrow = slice((h % 2) * 64, (h % 2) * 64 + 64)
                qT = qkT[h // 2]
                kT = qkT[KC + h // 2]
                av = ps_av.tile([VAUG, N], F32, name=f"av{h}", tag="av")
                for kc in range(TC):
                    ksl = slice(kc * P, (kc + 1) * P)
                    sc = ps_sc.tile([P, N], F32, name=f"sc{h}_{kc}", tag="sc")
                    for q in range(QC):
                        qsl = slice(q * 512, (q + 1) * 512)
                        nc.tensor.matmul(
                            sc[:, qsl], kT[hrow, ksl], qT[hrow, qsl],
                            start=True, stop=True,
                        )
                    eT = exp_pool.tile([P, N], FP16, name=f"e{h}_{kc}", tag="e")
                    nc.scalar.activation(
                        eT[:], sc[:], mybir.ActivationFunctionType.Exp
                    )
                    for q in range(QC):
                        qsl = slice(q * 512, (q + 1) * 512)
                        nc.tensor.matmul(
                            av[:, qsl],
                            v_aug[kc][:, h * VAUG : (h + 1) * VAUG],
                            eT[:, qsl],
                            start=(kc == 0), stop=(kc == TC - 1),
                        )
                    for fkc, fn, args in fillers[h]:
                        if fkc == kc:
                            fn(*args)
                if h == 8:
                    # wproj can land any time before the proj partials
                    for c in range(KC):
                        nc.sync.dma_start(
                            out=wproj[c][:],
                            in_=wproj_d[c * P : (c + 1) * P, :],
                        )
                # ---- normalization: sums -> 1/sums -> broadcast across
                # partitions via DRAM round trip -> DVE multiply ----
                sums_t = norm_pool.tile([1, N], F32, name=f"sums{h}", tag="sums")
                recip_t = norm_pool.tile([1, N], F32, name=f"recip{h}", tag="recip")
                nc.vector.tensor_copy(sums_t[:], av[64:65, :])
                nc.vector.reciprocal_approx_fast(out=recip_t[:], in_=sums_t[:])
                nc.sync.dma_start(out=recip_d[h : h + 1, :], in_=recip_t[:])
                rep = rep_pool.tile([64, N], F32, name=f"rep{h}", tag="rep")
                nc.sync.dma_start(
                    out=rep[:],
                    in_=recip_d[h : h + 1, :].to_broadcast([64, N]),
                )
                nc.vector.tensor_tensor(
                    out=outT[h // 2][hrow, :],
                    in0=av[0:64, :],
                    in1=rep[:],
                    op=mybir.AluOpType.mult,
                )

            # ---- epilogue: remaining proj partials bridge the last
            # head's normalization latency, then the c=5 finishes ----
            for t in range(TC - 3, TC):
                emit_proj_partial(t)
            for t in range(TC):
                emit_proj_finish(t)

    nc.compile()
    return nc


_NC_CACHE = {}


def kernel(**inputs) -> np.ndarray:
    x = np.asarray(inputs["x"], dtype=np.float32)
    qkv_w = np.asarray(inputs["qkv_w"], dtype=np.float32)
    qkv_b = np.asarray(inputs["qkv_b"], dtype=np.float32)
    proj_w = np.asarray(inputs["proj_w"], dtype=np.float32)
    proj_b = np.asarray(inputs["proj_b"], dtype=np.float32)
    # context is unused by the reference layer.

    scale = HEAD_DIM ** -0.5
    wqk = qkv_w[:, : 2 * DIM].copy()
    wqk[:, :DIM] *= scale
    wv = np.ascontiguousarray(qkv_w[:, 2 * DIM :])
    bqk = qkv_b[: 2 * DIM].copy()
    bqk[:DIM] *= scale
    bv = qkv_b[2 * DIM :].copy()

    with_qkv_bias = bool(np.any(qkv_b))
    with_proj_bias = bool(np.any(proj_b))

    key = (with_qkv_bias, with_proj_bias)
    if key not in _NC_CACHE:
        _NC_CACHE[key] = build_nc(*key)
    nc = _NC_CACHE[key]

    base = {
        "wqk": wqk.astype(np.float16),
        "wv": wv.astype(np.float16),
        "wproj": proj_w.astype(np.float16),
        "bqk": bqk.reshape(1, -1),
        "bv": bv.reshape(1, -1),
        "bproj": proj_b.reshape(1, -1),
    }
    in_maps = [
        {**base, "xT": np.ascontiguousarray(x[b].T).astype(np.float16)}
        for b in range(B)
    ]
    res = run_bass_kernel_spmd(nc, in_maps, list(range(B)))
    out = np.stack([res.results[b]["out"] for b in range(B)], axis=0)
    return out.astype(np.float32)
